# revision 19
# baseline (speedup 1.0000x reference)
"""Cross-attention Trainium2 kernel, 8-core data-parallel.

Problem (hardcoded): B=4, NQ=4096, NK=1024, QD=1024, CD=768, H=16, HD=64.
  out = softmax((x@Wq) @ (ctx@Wk)^T / sqrt(HD)) @ (ctx@Wv) @ Wo + bo

Sharding: pure data-parallel. 8 cores = 4 batches x 2 NQ-halves of 2048
query rows. Each core redundantly computes K/V projections for its batch
(cheap) and needs no collectives.

v4 design (v2 @409us; v3's DMA-XBAR transposes @641us showed the SP
queue serializing transposes against y/x DMAs, and 8 per-kc exps pushed
ACT to 99% busy -> 1.1us score stalls every hp):
 - PE issue queue is the bottleneck (pure work ~364us): scores 109us
   (contraction=64 is a hard 2x inefficiency), attn@V ~100us
   (LDWEIGHTS-bound: every exp output streams through the PE weight port
   at 128 elem/cycle), projections 150us.  ACT exp ~262us, NOT binding.
 - Score PSUM ring: one [128,3,1024] tile (6 banks), slot = running
   counter %3, one kc (=1024 f32: h0|h1) per slot.  exp flushes lazily:
   a pending pair of adjacent slots -> one 2048-wide exp; a slot-2
   single -> 1024-wide (5 exps/hp, ACT ~88% in-loop).  Reuse distance 3
   kc (~2-3us of PE with the vO/filler inserts) covers exp 1975ns+sem,
   killing v2's A/Bp-ring stalls.
 - attn@V emitted one qc at a time at 4 points inside the next hp's kc
   loop (kc 2/3/4/5); its PE transpose is lagged one insert and writes
   the spare cols [448:512] of the LIVE O psum slot, so transposes never
   allocate a psum ring slot (v2: 59us of transpose waits on the shared
   ACC ring).
 - ACC psum ring (2 banks) serves attn@V O slots + filler projections.
 - wk loads issue from the ACT queue in parallel with ctx on SP.
 - Output DRAM + y tiles in bf16 (halves tail DMA; 0.1% rounding, gate
   is 2e-2).  Drain interleaves y(t3) groups between vO(t3,hp7) chunks.
"""

import numpy as np

B, NQ, NK = 4, 4096, 1024
QD, CD, H = 1024, 768, 16
HD = QD // H
SCALE = HD ** -0.5
NQL = NQ // 2          # query rows per core
N_CORES = 8
W = 512                # query tile width
NT = NQL // W          # 4 query tiles
KC_Q = QD // 128       # 8
KC_C = CD // 128       # 6
NKC = NK // 128        # 8
HP = H // 2            # 8 head pairs


def build_bass():
    """Build the per-core Bass graph (SPMD, identical on all 8 cores)."""
    import concourse.bass as bass
    import concourse.tile as tile
    from concourse import bacc, mybir

    f32 = mybir.dt.float32
    bf16 = mybir.dt.bfloat16
    EXP = mybir.ActivationFunctionType.Exp

    nc = bacc.Bacc()

    xT_h = nc.dram_tensor("xT", (QD, NQL), bf16, kind="ExternalInput")
    ctxT_h = nc.dram_tensor("ctxT", (CD, NK), bf16, kind="ExternalInput")
    wq_h = nc.dram_tensor("wq", (QD, QD), bf16, kind="ExternalInput")
    wk_h = nc.dram_tensor("wk", (CD, QD), bf16, kind="ExternalInput")
    wv_h = nc.dram_tensor("wv", (CD, QD), bf16, kind="ExternalInput")
    wo_h = nc.dram_tensor("wo", (QD, QD), bf16, kind="ExternalInput")
    bo_h = nc.dram_tensor("bo", (1, QD), bf16, kind="ExternalInput")
    eye_h = nc.dram_tensor("eye", (128, 128), bf16, kind="ExternalInput")
    out_h = nc.dram_tensor("out", (NQL, QD), bf16, kind="ExternalOutput")

    xT_d = xT_h[:].rearrange("(c p) n -> p c n", p=128)       # [128, 8, 2048]
    ctxT_d = ctxT_h[:].rearrange("(c p) n -> p c n", p=128)   # [128, 6, 1024]
    wq_d = wq_h[:].rearrange("(c p) m -> p c m", p=128)
    wk_d = wk_h[:].rearrange("(c p) m -> p c m", p=128)
    wv_d = wv_h[:].rearrange("(c p) m -> p c m", p=128)
    wo_d = wo_h[:].rearrange("(c p) m -> p c m", p=128)
    out_d = out_h[:].rearrange("(t p) n -> p t n", p=128)     # [128, 16, 1024]

    with tile.TileContext(nc) as tc:
        _cms = []

        def open_pool(**kw):
            cm = tc.tile_pool(**kw)
            _cms.append(cm)
            return cm.__enter__()

        pp = open_pool(name="persist", bufs=1)
        # ---- persistent SBUF tiles
        qt_sb = pp.tile([128, KC_Q, 2, W], bf16)       # QT ring (per-tile)
        kt_sb = pp.tile([128, KC_Q, NK], bf16)         # KT   2 MB
        vp_sb = pp.tile([128, NKC, H, HD + 1], bf16)   # V'   2.08 MB
        attnT_sb = pp.tile([128, KC_Q, 2, W], bf16)    # attn ring (per-tile)
        wo_sb = pp.tile([128, KC_Q, QD], bf16)         # Wo   2 MB
        wq_sb = pp.tile([128, KC_Q, QD], bf16)         # Wq   2 MB
        # exp outputs: ring of 2 hp-buffers: slot index = (hp%2)*8 + kc
        etp = pp.tile([128, 16, 2, W], bf16)           # 4.2 MB
        bo_bc = pp.tile([128, QD], f32)
        ones_sb = pp.tile([1, 128], bf16)
        eye_sb = pp.tile([128, 128], bf16)

        # ---- PSUM (8 banks): score ring 6 + ACC ring 2
        psS = open_pool(name="psS", bufs=1, space=bass.MemorySpace.PSUM)
        S = psS.tile([128, 3, 1024], f32)
        accp = open_pool(name="accp", bufs=2, space=bass.MemorySpace.PSUM)

        # ---- small SBUF pools
        py = open_pool(name="yout", bufs=3)
        prs = open_pool(name="rsmall", bufs=2)
        pxq = open_pool(name="xq", bufs=2)
        pkv = open_pool(name="kvin", bufs=1)

        ctxT_sb = pkv.tile([128, KC_C, NK], bf16)      # 1.5 MB
        wk_sb = pkv.tile([128, KC_C, QD], bf16)        # 1.5 MB
        wv_sb = pkv.tile([128, KC_C, QD], bf16)        # 1.5 MB
        bo_sb = pkv.tile([1, QD], bf16)

        # copy engines for PSUM->SBUF moves; scalar only in the prefix
        _cp_idx = [0]

        def copy_out(dst, src, in_attention):
            # gpsimd cannot read PSUM; scalar must stay free during attention
            engs = (nc.vector,) if in_attention else (nc.vector, nc.scalar)
            eng = engs[_cp_idx[0] % len(engs)]
            _cp_idx[0] += 1
            if eng is nc.scalar:
                eng.copy(dst, src)
            else:
                eng.tensor_copy(dst, src)

        # ---------------- prefix ----------------
        # ctx on the SP queue, wk/wv heads on the ACT queue: the DMA
        # streams issue concurrently so K-proj's gate is max() not sum().
        nc.sync.dma_start(ctxT_sb[:, :, 0:512], ctxT_d[:, :, 0:512])
        nc.scalar.dma_start(wk_sb[:, :, 0:256], wk_d[:, :, 0:256])
        nc.scalar.dma_start(wv_sb[:, :, 0:256], wv_d[:, :, 0:256])
        nc.scalar.dma_start(bo_sb[:], bo_h[:])
        nc.scalar.dma_start(eye_sb[:], eye_h[:])
        nc.sync.dma_start(ctxT_sb[:, :, 512:1024], ctxT_d[:, :, 512:1024])

        xt0 = pxq.tile([128, KC_Q, W], bf16, tag="xt", name="xt0")
        nc.sync.dma_start(xt0[:], xT_d[:, :, 0:W])
        nc.sync.dma_start(wq_sb[:, :, 0:256], wq_d[:, :, 0:256])
        nc.scalar.dma_start(wk_sb[:, :, 256:1024], wk_d[:, :, 256:1024])
        nc.scalar.dma_start(wv_sb[:, :, 256:1024], wv_d[:, :, 256:1024])
        nc.sync.dma_start(wq_sb[:, :, 256:1024], wq_d[:, :, 256:1024])

        nc.vector.memset(ones_sb[:], 1.0)
        nc.vector.memset(vp_sb[:, :, :, HD], 1.0)    # ones column per head
        for no in range(2):
            ps = accp.tile([128, 512], f32, tag="acc", name="psb")
            nc.tensor.matmul(ps[:], ones_sb[:],
                             bo_sb[0:1, no * 512:(no + 1) * 512],
                             start=True, stop=True)
            nc.scalar.copy(bo_bc[:, no * 512:(no + 1) * 512], ps[:])

        def k_proj(mo, in_attention):
            for nk in range(2):
                ps = accp.tile([128, 512], f32, tag="acc", name="psk")
                for c in range(KC_C):
                    nc.tensor.matmul(
                        ps[:],
                        wk_sb[:, c, mo * 128:(mo + 1) * 128],
                        ctxT_sb[:, c, nk * 512:(nk + 1) * 512],
                        start=(c == 0), stop=(c == KC_C - 1),
                    )
                copy_out(kt_sb[:, mo, nk * 512:(nk + 1) * 512], ps[:],
                         in_attention)

        def v_proj2(ko0, hg, in_attention):
            """V' for heads (2hg, 2hg+1), key chunks ko0..ko0+3."""
            for ko in range(ko0, ko0 + 4):
                ps = accp.tile([128, 512], f32, tag="acc", name="psv")
                for c in range(KC_C):
                    nc.tensor.matmul(
                        ps[:, 0:128],
                        ctxT_sb[:, c, ko * 128:(ko + 1) * 128],
                        wv_sb[:, c, hg * 128:(hg + 1) * 128],
                        start=(c == 0), stop=(c == KC_C - 1),
                    )
                copy_out(
                    vp_sb[:, ko, 2 * hg:2 * hg + 2, 0:HD],
                    ps[:, 0:128].rearrange("p (h d) -> p h d", h=2),
                    in_attention)

        def q_proj(t, xt, mo, in_attention):
            ps = accp.tile([128, 512], f32, tag="acc", name="psq")
            for c in range(KC_Q):
                nc.tensor.matmul(
                    ps[:],
                    wq_sb[:, c, mo * 128:(mo + 1) * 128],
                    xt[:, c, :],
                    start=(c == 0), stop=(c == KC_Q - 1),
                )
            copy_out(qt_sb[:, mo, t % 2, :], ps[:], in_attention)

        # prefix compute: only what tile0's first head-pairs need.
        k_proj(0, False)
        k_proj(1, False)
        v_proj2(0, 0, False)
        v_proj2(4, 0, False)
        for mo in range(KC_Q):
            q_proj(0, xt0, mo, False)

        nc.sync.dma_start(wo_sb[:], wo_d)

        # ---------- filler generators (yield once per matmul group) ----
        def t0_group():
            # K for mo>=2 pulled 2 head-pairs before scores need them;
            # V head-group hg pulled >=1 hp before attn@V(hp=hg) reads it.
            for mo in range(2, KC_Q):
                k_proj(mo, True)
                yield
                v_proj2(0, mo - 1, True)
                yield
                v_proj2(4, mo - 1, True)
                yield
            v_proj2(0, 7, True)
            yield
            v_proj2(4, 7, True)
            yield

        def q_group(t, xt):
            for mo in range(KC_Q):
                q_proj(t, xt, mo, True)
                yield

        def y_group(t):
            for lo in range(4):
                mo = t * 4 + lo
                for no in range(2):
                    ps = accp.tile([128, 512], f32, tag="acc", name="psy")
                    for c in range(KC_Q):
                        nc.tensor.matmul(
                            ps[:],
                            attnT_sb[:, c, t % 2, lo * 128:(lo + 1) * 128],
                            wo_sb[:, c, no * 512:(no + 1) * 512],
                            start=(c == 0), stop=(c == KC_Q - 1),
                        )
                    y = py.tile([128, 512], bf16, tag="y")
                    nc.vector.tensor_add(
                        y[:], ps[:], bo_bc[:, no * 512:(no + 1) * 512])
                    nc.sync.dma_start(
                        out_d[:, mo, no * 512:(no + 1) * 512], y[:])
                    yield

        # ---------------- main attention loop ----------------
        pst = open_pool(name="stg", bufs=12)

        # one normalized stage awaiting its PE transpose: (t, hp, qc, stage)
        _tp_pend = [None]

        def do_transpose(O):
            """Transpose the pending stage into O's spare cols [448:512]
            (f32 view; 128 bf16 cols), then DVE-copy to attnT.  Piggybacks
            on the live O slot so transposes never contend for a psum ring
            slot (v2's 59us of transpose waits)."""
            if _tp_pend[0] is None:
                return
            pt, php, pqc, pstage = _tp_pend[0]
            _tp_pend[0] = None
            tp16 = O[:, 448:512].bitcast(bf16)
            nc.tensor.transpose(tp16, pstage[:], eye_sb[:])
            nc.vector.tensor_copy(
                attnT_sb[:, php, pt % 2, pqc * 128:(pqc + 1) * 128], tp16)

        def attn_vO(t, hp, qc):
            """attn@V in O[q,d] form + normalize into an SBUF stage tile.

            One PSUM slot holds both heads: O[:,0:65]=h0 (col 64 = softmax
            denominator), O[:,65:130]=h1.  Normalize with a per-partition
            reciprocal + tensor_scalar into stage[q,d] (bf16).  The PE
            transpose back to attnT's [d,q] layout happens one insert
            later (stage's DVE chain is then long done -> no PE wait).
            """
            r = hp % 2
            O = accp.tile([128, 512], f32, tag="acc", name="O")
            for h_i in (0, 1):
                h = 2 * hp + h_i
                dst = O[:, h_i * 65:h_i * 65 + 65]
                for kc in range(NKC):
                    nc.tensor.matmul(
                        dst,
                        etp[:, r * 8 + kc, h_i, qc * 128:(qc + 1) * 128],
                        vp_sb[:, kc, h, :],
                        start=(kc == 0), stop=(kc == NKC - 1),
                    )
            do_transpose(O)
            stage = pst.tile([128, 128], bf16, tag="st")
            for h_i in (0, 1):
                rcp = prs.tile([128, 1], f32, tag="rcp")
                nc.vector.reciprocal_approx_fast(
                    rcp[:], O[:, h_i * 65 + 64:h_i * 65 + 65])
                nc.vector.tensor_scalar_mul(
                    stage[:, h_i * 64:(h_i + 1) * 64],
                    O[:, h_i * 65:h_i * 65 + 64], rcp[:])
            _tp_pend[0] = (t, hp, qc, stage)

        pending = None          # (t, hp) owed attn@V
        fillers = []

        def take_filler(n):
            # drain sequentially: first generator until exhausted, then next
            for _ in range(n):
                while fillers:
                    if next(fillers[0], "DONE") == "DONE":
                        fillers.pop(0)
                        continue
                    break

        # filler pulls per hp: tile0 drains the folded-in K/V projections
        # (one mo-group per hp, 2 head-pairs of margin) then q(1); later
        # tiles drain q(t+1) (hp0-3) then y(t-1) (hp4-7, which keeps
        # y(t-1) clear of hp0 until vO(t-1,hp7)'s transposes land); t3
        # delays y(2) to hp2 for the same reason (no q pulls ahead of it).
        PULL_SCHED = {
            0: [4, 4, 4, 4, 3, 3, 3, 3],
            1: [2, 2, 2, 2, 2, 2, 2, 2],
            2: [2, 2, 2, 2, 2, 2, 2, 2],
            3: [0, 0, 2, 2, 2, 2, 2, 2],
        }

        # score psum ring: running slot counter (%3); one 1024-wide exp
        # per kc.  The ring WAR chain then has lag 3 (exp(kc) gates only
        # spair(kc+3)) so ACT throughput, not the sem round-trip, paces
        # the loop.
        _slot = [0]

        for t in range(NT):
            if t == 0:
                fillers.append(t0_group())
            if t + 1 < NT:
                xt = pxq.tile([128, KC_Q, W], bf16, tag="xt",
                              name=f"xt{t + 1}")
                nc.sync.dma_start(xt[:], xT_d[:, :, (t + 1) * W:(t + 2) * W])
                fillers.append(q_group(t + 1, xt))
            if t > 0:
                fillers.append(y_group(t - 1))

            for hp in range(HP):
                r = hp % 2
                quota = PULL_SCHED[t][hp]
                p1 = (quota + 2) // 3
                p2 = (quota - p1 + 1) // 2
                p3 = quota - p1 - p2

                for kc in range(NKC):
                    s = _slot[0]
                    _slot[0] = (s + 1) % 3
                    ks = slice(kc * 128, (kc + 1) * 128)
                    nc.tensor.matmul(
                        S[:, s, 0:512], kt_sb[0:64, hp, ks],
                        qt_sb[0:64, hp, t % 2, :],
                        start=True, stop=True, tile_position=(0, 0),
                    )
                    nc.tensor.matmul(
                        S[:, s, 512:1024], kt_sb[64:128, hp, ks],
                        qt_sb[64:128, hp, t % 2, :],
                        start=True, stop=True, tile_position=(64, 0),
                    )
                    nc.scalar.activation(
                        etp[:, r * 8 + kc, :, :], S[:, s, :],
                        EXP, scale=SCALE)
                    # attn@V of the previous hp spread between kcs;
                    # filler pulls at kc 1/4/7
                    if kc == 1:
                        take_filler(p1)
                    elif kc == 4:
                        take_filler(p2)
                    elif kc == 7:
                        take_filler(p3)
                    if pending is not None:
                        if kc == 2:
                            attn_vO(*pending, 0)
                        elif kc == 3:
                            attn_vO(*pending, 1)
                        elif kc == 5:
                            attn_vO(*pending, 2)
                        elif kc == 6:
                            attn_vO(*pending, 3)
                pending = (t, hp)

        # drain: last head-pair's attn@V interleaved with the last y tile
        yg = y_group(NT - 1)
        take_filler(1000)
        attn_vO(*pending, 0)
        attn_vO(*pending, 1)     # transposes qc0
        next(yg)  # lo0,no0 (reads attnT c7 cols 0:128 <- qc0)
        next(yg)  # lo0,no1
        attn_vO(*pending, 2)     # transposes qc1
        next(yg)  # lo1,no0
        next(yg)
        attn_vO(*pending, 3)     # transposes qc2
        Ofl = accp.tile([128, 512], f32, tag="acc", name="Ofl")
        do_transpose(Ofl)        # flush qc3
        for _ in yg:
            pass

        for cm in reversed(_cms):
            cm.__exit__(None, None, None)

    nc.finalize()
    return nc


def make_in_maps(x, context, Wq, Wk, Wv, Wo, bo):
    """Host-side sharding + layout prep: transpose and cast to bf16."""
    import ml_dtypes
    bf16 = ml_dtypes.bfloat16

    x = np.asarray(x, np.float32)
    context = np.asarray(context, np.float32)
    wq = np.asarray(Wq, np.float32).astype(bf16)
    wk = np.asarray(Wk, np.float32).astype(bf16)
    wv = np.asarray(Wv, np.float32).astype(bf16)
    wo = np.asarray(Wo, np.float32).astype(bf16)
    bo = np.asarray(bo, np.float32).reshape(1, QD).astype(bf16)
    eye = np.eye(128, dtype=np.float32).astype(bf16)

    in_maps = []
    for c in range(N_CORES):
        b, half = c // 2, c % 2
        xs = x[b, half * NQL:(half + 1) * NQL, :]           # [2048, 1024]
        in_maps.append({
            "xT": np.ascontiguousarray(xs.T).astype(bf16),   # [1024, 2048]
            "ctxT": np.ascontiguousarray(context[b].T).astype(bf16),
            "wq": wq, "wk": wk, "wv": wv, "wo": wo, "bo": bo, "eye": eye,
        })
    return in_maps


_NC_CACHE = {}


def kernel(x, context, Wq, Wk, Wv, Wo, bo, _trace=False):
    import sys
    if "/opt/trn_rl_repo" not in sys.path:
        sys.path.insert(0, "/opt/trn_rl_repo")
    from concourse.bass_utils import run_bass_kernel_spmd

    if "nc" not in _NC_CACHE:
        _NC_CACHE["nc"] = build_bass()
    nc = _NC_CACHE["nc"]

    in_maps = make_in_maps(x, context, Wq, Wk, Wv, Wo, bo)
    res = run_bass_kernel_spmd(
        nc, in_maps, core_ids=list(range(N_CORES)), trace=_trace)

    out = np.empty((B, NQ, QD), np.float32)
    for c in range(N_CORES):
        b, half = c // 2, c % 2
        out[b, half * NQL:(half + 1) * NQL, :] = np.asarray(
            res.results[c]["out"]).astype(np.float32)
    if _trace:
        return out, res
    return out


# revision 20
# speedup vs baseline: 1.3432x; 1.3432x over previous
"""Cross-attention Trainium2 kernel, 8-core data-parallel.

Problem (hardcoded): B=4, NQ=4096, NK=1024, QD=1024, CD=768, H=16, HD=64.
  out = softmax((x@Wq) @ (ctx@Wk)^T / sqrt(HD)) @ (ctx@Wv) @ Wo + bo

Sharding: pure data-parallel. 8 cores = 4 batches x 2 NQ-halves of 2048
query rows. Each core redundantly computes K/V projections for its batch
(cheap) and needs no collectives.

v6 = the empirically-balanced v2 schedule (A/Bp exp psum rings, 2048/
1024-wide exps, 2-qc attn@V bursts, N_FILL filler slots) plus exactly
three fixes that each measured clean in isolation:
 - attn@V's PE transpose is issued one vO-call later and writes the
   spare cols [448:512] of the then-live O psum slot, so transposes no
   longer allocate from the shared ACC ring (v2: 59us of transpose
   waits) and their stage input is long-normalized (no DVE wait).
 - Prefix DMAs split across the SP (ctx/x/wq/wo) and ACT (wk/wv/bo/eye)
   queues: K-proj's gate drops from 14us to ~7us.
 - Output DRAM + y tiles in bf16 (halves tail DMA; 0.1% rounding,
   gate is 2e-2).  Drain interleaves y(t3) with the vO(t3,hp7) chunks.

Rejected variants (all measured SLOWER end-to-end on HW): DMA-XBAR
transposes (Sync-queue serialization, 641us), per-kc 1024-wide exps on
a rotating 3-slot ring (ring-reuse stalls every hp; PE idle drops the
clock to pstate-mid, 516-558us), folding the K/V/Q prefix into the
loop as fillers (amplified the same stalls).
"""

import numpy as np

B, NQ, NK = 4, 4096, 1024
QD, CD, H = 1024, 768, 16
HD = QD // H
SCALE = HD ** -0.5
NQL = NQ // 2          # query rows per core
N_CORES = 8
W = 512                # query tile width
NT = NQL // W          # 4 query tiles
KC_Q = QD // 128       # 8
KC_C = CD // 128       # 6
NKC = NK // 128        # 8
HP = H // 2            # 8 head pairs


def build_bass():
    """Build the per-core Bass graph (SPMD, identical on all 8 cores)."""
    import concourse.bass as bass
    import concourse.tile as tile
    from concourse import bacc, mybir

    f32 = mybir.dt.float32
    bf16 = mybir.dt.bfloat16
    EXP = mybir.ActivationFunctionType.Exp

    nc = bacc.Bacc()

    xT_h = nc.dram_tensor("xT", (QD, NQL), bf16, kind="ExternalInput")
    ctxT_h = nc.dram_tensor("ctxT", (CD, NK), bf16, kind="ExternalInput")
    wq_h = nc.dram_tensor("wq", (QD, QD), bf16, kind="ExternalInput")
    wk_h = nc.dram_tensor("wk", (CD, QD), bf16, kind="ExternalInput")
    wv_h = nc.dram_tensor("wv", (CD, QD), bf16, kind="ExternalInput")
    wo_h = nc.dram_tensor("wo", (QD, QD), bf16, kind="ExternalInput")
    bo_h = nc.dram_tensor("bo", (1, QD), bf16, kind="ExternalInput")
    eye_h = nc.dram_tensor("eye", (128, 128), bf16, kind="ExternalInput")
    out_h = nc.dram_tensor("out", (NQL, QD), bf16, kind="ExternalOutput")

    xT_d = xT_h[:].rearrange("(c p) n -> p c n", p=128)       # [128, 8, 2048]
    ctxT_d = ctxT_h[:].rearrange("(c p) n -> p c n", p=128)   # [128, 6, 1024]
    wq_d = wq_h[:].rearrange("(c p) m -> p c m", p=128)
    wk_d = wk_h[:].rearrange("(c p) m -> p c m", p=128)
    wv_d = wv_h[:].rearrange("(c p) m -> p c m", p=128)
    wo_d = wo_h[:].rearrange("(c p) m -> p c m", p=128)
    out_d = out_h[:].rearrange("(t p) n -> p t n", p=128)     # [128, 16, 1024]

    with tile.TileContext(nc) as tc:
        _cms = []

        def open_pool(**kw):
            cm = tc.tile_pool(**kw)
            _cms.append(cm)
            return cm.__enter__()

        pp = open_pool(name="persist", bufs=1)
        # ---- persistent SBUF tiles
        qt_sb = pp.tile([128, KC_Q, 2, W], bf16)       # QT ring (per-tile)
        kt_sb = pp.tile([128, KC_Q, NK], bf16)         # KT   2 MB
        vp_sb = pp.tile([128, NKC, H, HD + 1], bf16)   # V'   2.08 MB
        attnT_sb = pp.tile([128, KC_Q, 2, W], bf16)    # attn ring (per-tile)
        wo_sb = pp.tile([128, KC_Q, QD], bf16)         # Wo   2 MB
        wq_sb = pp.tile([128, KC_Q, QD], bf16)         # Wq   2 MB
        # exp outputs: ring of 2 hp-buffers: slot index = (hp%2)*8 + kc
        etp = pp.tile([128, 16, 2, W], bf16)           # 4.2 MB
        bo_bc = pp.tile([128, QD], f32)
        ones_sb = pp.tile([1, 128], bf16)
        eye_sb = pp.tile([128, 128], bf16)

        # ---- PSUM (8 banks): A 4 + Bp 2 + ACC ring 2
        psS = open_pool(name="psS", bufs=1, space=bass.MemorySpace.PSUM)
        A = psS.tile([128, 2, 1024], f32)
        Bp = psS.tile([128, 1024], f32)
        accp = open_pool(name="accp", bufs=2, space=bass.MemorySpace.PSUM)

        # ---- small SBUF pools
        py = open_pool(name="yout", bufs=3)
        prs = open_pool(name="rsmall", bufs=2)
        pxq = open_pool(name="xq", bufs=2)
        pkv = open_pool(name="kvin", bufs=1)

        ctxT_sb = pkv.tile([128, KC_C, NK], bf16)      # 1.5 MB
        wk_sb = pkv.tile([128, KC_C, QD], bf16)        # 1.5 MB
        wv_sb = pkv.tile([128, KC_C, QD], bf16)        # 1.5 MB
        bo_sb = pkv.tile([1, QD], bf16)

        # copy engines for PSUM->SBUF moves; scalar only in the prefix
        _cp_idx = [0]

        def copy_out(dst, src, in_attention):
            # gpsimd cannot read PSUM; scalar must stay free during attention
            engs = (nc.vector,) if in_attention else (nc.vector, nc.scalar)
            eng = engs[_cp_idx[0] % len(engs)]
            _cp_idx[0] += 1
            if eng is nc.scalar:
                eng.copy(dst, src)
            else:
                eng.tensor_copy(dst, src)

        # ---------------- prefix ----------------
        # ctx/x/wq on the SP queue, wk/wv/bo/eye on the ACT queue: the
        # two DMA streams flow concurrently so K-proj's gate shrinks.
        nc.sync.dma_start(ctxT_sb[:, :, 0:512], ctxT_d[:, :, 0:512])
        nc.scalar.dma_start(wk_sb[:, :, 0:256], wk_d[:, :, 0:256])
        nc.scalar.dma_start(bo_sb[:], bo_h[:])
        nc.scalar.dma_start(eye_sb[:], eye_h[:])
        nc.sync.dma_start(ctxT_sb[:, :, 512:1024], ctxT_d[:, :, 512:1024])
        nc.scalar.dma_start(wk_sb[:, :, 256:1024], wk_d[:, :, 256:1024])

        xt0 = pxq.tile([128, KC_Q, W], bf16, tag="xt", name="xt0")

        nc.vector.memset(ones_sb[:], 1.0)
        nc.vector.memset(vp_sb[:, :, :, HD], 1.0)    # ones column per head
        for no in range(2):
            ps = accp.tile([128, 512], f32, tag="acc", name="psb")
            nc.tensor.matmul(ps[:], ones_sb[:],
                             bo_sb[0:1, no * 512:(no + 1) * 512],
                             start=True, stop=True)
            nc.scalar.copy(bo_bc[:, no * 512:(no + 1) * 512], ps[:])

        # ---- K projection (all keys; needed before any scores)
        for mo in range(KC_Q):
            for nk in range(2):
                ps = accp.tile([128, 512], f32, tag="acc", name="psk")
                for c in range(KC_C):
                    nc.tensor.matmul(
                        ps[:],
                        wk_sb[:, c, mo * 128:(mo + 1) * 128],
                        ctxT_sb[:, c, nk * 512:(nk + 1) * 512],
                        start=(c == 0), stop=(c == KC_C - 1),
                    )
                copy_out(kt_sb[:, mo, nk * 512:(nk + 1) * 512], ps[:], False)
            if mo == 0:
                nc.scalar.dma_start(wv_sb[:], wv_d)
                nc.sync.dma_start(xt0[:], xT_d[:, :, 0:W])
                nc.sync.dma_start(wq_sb[:], wq_d)

        def v_proj(ko, nv, in_attention):
            ps = accp.tile([128, 512], f32, tag="acc", name="psv")
            for c in range(KC_C):
                nc.tensor.matmul(
                    ps[:],
                    ctxT_sb[:, c, ko * 128:(ko + 1) * 128],
                    wv_sb[:, c, nv * 512:(nv + 1) * 512],
                    start=(c == 0), stop=(c == KC_C - 1),
                )
            copy_out(
                vp_sb[:, ko, nv * 8:(nv + 1) * 8, 0:HD],
                ps[:].rearrange("p (h d) -> p h d", h=8), in_attention)

        # ---- V projection for heads 0-7 (needed by tile0's early attn@V)
        for ko in range(NKC):
            v_proj(ko, 0, False)

        # ---- Q projection for tile 0
        for mo in range(KC_Q):
            ps = accp.tile([128, 512], f32, tag="acc", name="psq")
            for c in range(KC_Q):
                nc.tensor.matmul(
                    ps[:],
                    wq_sb[:, c, mo * 128:(mo + 1) * 128],
                    xt0[:, c, :],
                    start=(c == 0), stop=(c == KC_Q - 1),
                )
            copy_out(qt_sb[:, mo, 0, :], ps[:], False)

        nc.sync.dma_start(wo_sb[:], wo_d)

        # ---------- filler generators (yield once per matmul group) ----
        # V heads 8-15: drained during tile0 hp0-3, strictly before any
        # attn@V of hp>=4 (which is emitted at hp>=5) reads them.
        def v_group():
            for ko in range(NKC):
                v_proj(ko, 1, True)
                yield

        def q_group(t, xt):
            for mo in range(KC_Q):
                ps = accp.tile([128, 512], f32, tag="acc", name="psq2")
                for c in range(KC_Q):
                    nc.tensor.matmul(
                        ps[:],
                        wq_sb[:, c, mo * 128:(mo + 1) * 128],
                        xt[:, c, :],
                        start=(c == 0), stop=(c == KC_Q - 1),
                    )
                copy_out(qt_sb[:, mo, t % 2, :], ps[:], True)
                yield

        def y_group(t):
            for lo in range(4):
                mo = t * 4 + lo
                for no in range(2):
                    ps = accp.tile([128, 512], f32, tag="acc", name="psy")
                    for c in range(KC_Q):
                        nc.tensor.matmul(
                            ps[:],
                            attnT_sb[:, c, t % 2, lo * 128:(lo + 1) * 128],
                            wo_sb[:, c, no * 512:(no + 1) * 512],
                            start=(c == 0), stop=(c == KC_Q - 1),
                        )
                    y = py.tile([128, 512], bf16, tag="y")
                    nc.vector.tensor_add(
                        y[:], ps[:], bo_bc[:, no * 512:(no + 1) * 512])
                    nc.sync.dma_start(
                        out_d[:, mo, no * 512:(no + 1) * 512], y[:])
                    yield

        # ---------------- main attention loop ----------------
        pst = open_pool(name="stg", bufs=12)

        # one normalized stage awaiting its PE transpose: (t, hp, qc, stage)
        _tp_pend = [None]

        def do_transpose(O):
            """Transpose the pending stage into O's spare cols [448:512]
            (f32 view; 128 bf16 cols), then DVE-copy to attnT.  Piggybacks
            on the live O slot so transposes never contend for a psum ring
            slot, and the lag means the stage's DVE chain is already done
            when the PE reaches the transpose."""
            if _tp_pend[0] is None:
                return
            pt, php, pqc, pstage = _tp_pend[0]
            _tp_pend[0] = None
            tp16 = O[:, 448:512].bitcast(bf16)
            nc.tensor.transpose(tp16, pstage[:], eye_sb[:])
            nc.vector.tensor_copy(
                attnT_sb[:, php, pt % 2, pqc * 128:(pqc + 1) * 128], tp16)

        def attn_vO(t, hp, qcs):
            """attn@V in O[q,d] form + normalize into SBUF stage tiles.

            One PSUM slot holds both heads: O[:,0:65]=h0 (col 64 = softmax
            denominator), O[:,65:130]=h1.  Normalize with a per-partition
            reciprocal + tensor_scalar into stage[q,d] (bf16).
            """
            r = hp % 2
            for qc in qcs:
                O = accp.tile([128, 512], f32, tag="acc", name="O")
                for h_i in (0, 1):
                    h = 2 * hp + h_i
                    dst = O[:, h_i * 65:h_i * 65 + 65]
                    for kc in range(NKC):
                        nc.tensor.matmul(
                            dst,
                            etp[:, r * 8 + kc, h_i, qc * 128:(qc + 1) * 128],
                            vp_sb[:, kc, h, :],
                            start=(kc == 0), stop=(kc == NKC - 1),
                        )
                do_transpose(O)
                stage = pst.tile([128, 128], bf16, tag="st")
                for h_i in (0, 1):
                    rcp = prs.tile([128, 1], f32, tag="rcp")
                    nc.vector.reciprocal_approx_fast(
                        rcp[:], O[:, h_i * 65 + 64:h_i * 65 + 65])
                    nc.vector.tensor_scalar_mul(
                        stage[:, h_i * 64:(h_i + 1) * 64],
                        O[:, h_i * 65:h_i * 65 + 64], rcp[:])
                _tp_pend[0] = (t, hp, qc, stage)

        pending = None          # (t, hp) owed attn@V
        fillers = []

        def take_filler(n):
            # drain sequentially: first generator until exhausted, then next
            for _ in range(n):
                while fillers:
                    if next(fillers[0], "DONE") == "DONE":
                        fillers.pop(0)
                        continue
                    break

        # y(t-1) must not drain at hp0: its c=7 matmul needs the transpose
        # of (t-1, hp7) which is only issued while hp0 runs.
        N_FILL = [0, 2, 2, 2, 2, 2, 3, 3]

        for t in range(NT):
            if t == 0:
                fillers.append(v_group())
            else:
                fillers.append(y_group(t - 1))
            if t + 1 < NT:
                xt = pxq.tile([128, KC_Q, W], bf16, tag="xt",
                              name=f"xt{t + 1}")
                nc.sync.dma_start(xt[:], xT_d[:, :, (t + 1) * W:(t + 2) * W])
                fillers.append(q_group(t + 1, xt))

            for hp in range(HP):
                r = hp % 2

                def spair(kc, dst_h0, dst_h1):
                    ks = slice(kc * 128, (kc + 1) * 128)
                    nc.tensor.matmul(
                        dst_h0, kt_sb[0:64, hp, ks],
                        qt_sb[0:64, hp, t % 2, :],
                        start=True, stop=True, tile_position=(0, 0),
                    )
                    nc.tensor.matmul(
                        dst_h1, kt_sb[64:128, hp, ks],
                        qt_sb[64:128, hp, t % 2, :],
                        start=True, stop=True, tile_position=(64, 0),
                    )

                def exp_a(kc):
                    nc.scalar.activation(
                        etp[:, r * 8 + kc:r * 8 + kc + 2, :, :],
                        A[:], EXP, scale=SCALE)

                def exp_b(kc):
                    nc.scalar.activation(
                        etp[:, r * 8 + kc:r * 8 + kc + 1, :, :],
                        Bp[:], EXP, scale=SCALE)

                # kc pattern: {0,1}->A, 2->B, {3,4}->A, 5->B, {6,7}->A
                spair(0, A[:, 0, 0:512], A[:, 0, 512:1024])
                spair(1, A[:, 1, 0:512], A[:, 1, 512:1024])
                exp_a(0)
                spair(2, Bp[:, 0:512], Bp[:, 512:1024])
                exp_b(2)
                if pending is not None:
                    attn_vO(*pending, (0, 1))
                spair(3, A[:, 0, 0:512], A[:, 0, 512:1024])
                spair(4, A[:, 1, 0:512], A[:, 1, 512:1024])
                exp_a(3)
                if pending is not None:
                    attn_vO(*pending, (2,))
                spair(5, Bp[:, 0:512], Bp[:, 512:1024])
                exp_b(5)
                if pending is not None:
                    attn_vO(*pending, (3,))
                spair(6, A[:, 0, 0:512], A[:, 0, 512:1024])
                spair(7, A[:, 1, 0:512], A[:, 1, 512:1024])
                exp_a(6)
                take_filler(N_FILL[hp])
                pending = (t, hp)

        # drain: last head-pair's attn@V interleaved with the last y tile
        yg = y_group(NT - 1)
        take_filler(1000)
        attn_vO(*pending, (0,))
        attn_vO(*pending, (1,))     # transposes qc0
        next(yg)  # lo0,no0 (reads attnT c7 cols 0:128 <- qc0)
        next(yg)  # lo0,no1
        attn_vO(*pending, (2,))     # transposes qc1
        next(yg)  # lo1,no0
        next(yg)
        attn_vO(*pending, (3,))     # transposes qc2
        Ofl = accp.tile([128, 512], f32, tag="acc", name="Ofl")
        do_transpose(Ofl)           # flush qc3
        for _ in yg:
            pass

        for cm in reversed(_cms):
            cm.__exit__(None, None, None)

    nc.finalize()
    return nc


def make_in_maps(x, context, Wq, Wk, Wv, Wo, bo):
    """Host-side sharding + layout prep: transpose and cast to bf16."""
    import ml_dtypes
    bf16 = ml_dtypes.bfloat16

    x = np.asarray(x, np.float32)
    context = np.asarray(context, np.float32)
    wq = np.asarray(Wq, np.float32).astype(bf16)
    wk = np.asarray(Wk, np.float32).astype(bf16)
    wv = np.asarray(Wv, np.float32).astype(bf16)
    wo = np.asarray(Wo, np.float32).astype(bf16)
    bo = np.asarray(bo, np.float32).reshape(1, QD).astype(bf16)
    eye = np.eye(128, dtype=np.float32).astype(bf16)

    in_maps = []
    for c in range(N_CORES):
        b, half = c // 2, c % 2
        xs = x[b, half * NQL:(half + 1) * NQL, :]           # [2048, 1024]
        in_maps.append({
            "xT": np.ascontiguousarray(xs.T).astype(bf16),   # [1024, 2048]
            "ctxT": np.ascontiguousarray(context[b].T).astype(bf16),
            "wq": wq, "wk": wk, "wv": wv, "wo": wo, "bo": bo, "eye": eye,
        })
    return in_maps


_NC_CACHE = {}


def kernel(x, context, Wq, Wk, Wv, Wo, bo, _trace=False):
    import sys
    if "/opt/trn_rl_repo" not in sys.path:
        sys.path.insert(0, "/opt/trn_rl_repo")
    from concourse.bass_utils import run_bass_kernel_spmd

    if "nc" not in _NC_CACHE:
        _NC_CACHE["nc"] = build_bass()
    nc = _NC_CACHE["nc"]

    in_maps = make_in_maps(x, context, Wq, Wk, Wv, Wo, bo)
    res = run_bass_kernel_spmd(
        nc, in_maps, core_ids=list(range(N_CORES)), trace=_trace)

    out = np.empty((B, NQ, QD), np.float32)
    for c in range(N_CORES):
        b, half = c // 2, c % 2
        out[b, half * NQL:(half + 1) * NQL, :] = np.asarray(
            res.results[c]["out"]).astype(np.float32)
    if _trace:
        return out, res
    return out


# revision 22
# speedup vs baseline: 1.3589x; 1.0116x over previous
"""Cross-attention Trainium2 kernel, 8-core data-parallel.

Problem (hardcoded): B=4, NQ=4096, NK=1024, QD=1024, CD=768, H=16, HD=64.
  out = softmax((x@Wq) @ (ctx@Wk)^T / sqrt(HD)) @ (ctx@Wv) @ Wo + bo

Sharding: pure data-parallel. 8 cores = 4 batches x 2 NQ-halves of 2048
query rows. Each core redundantly computes K/V projections for its batch
(cheap) and needs no collectives.

v6 = the empirically-balanced v2 schedule (A/Bp exp psum rings, 2048/
1024-wide exps, 2-qc attn@V bursts, N_FILL filler slots) plus exactly
three fixes that each measured clean in isolation:
 - attn@V's PE transpose is issued one vO-call later and writes the
   spare cols [448:512] of the then-live O psum slot, so transposes no
   longer allocate from the shared ACC ring (v2: 59us of transpose
   waits) and their stage input is long-normalized (no DVE wait).
 - Prefix DMAs split across the SP (ctx/x/wq/wo) and ACT (wk/wv/bo/eye)
   queues: K-proj's gate drops from 14us to ~7us.
 - Output DRAM + y tiles in bf16 (halves tail DMA; 0.1% rounding,
   gate is 2e-2).  Drain interleaves y(t3) with the vO(t3,hp7) chunks.

Rejected variants (all measured SLOWER end-to-end on HW): DMA-XBAR
transposes (Sync-queue serialization, 641us), per-kc 1024-wide exps on
a rotating 3-slot ring (ring-reuse stalls every hp; PE idle drops the
clock to pstate-mid, 516-558us), folding the K/V/Q prefix into the
loop as fillers (amplified the same stalls).
"""

import numpy as np

B, NQ, NK = 4, 4096, 1024
QD, CD, H = 1024, 768, 16
HD = QD // H
SCALE = HD ** -0.5
NQL = NQ // 2          # query rows per core
N_CORES = 8
W = 512                # query tile width
NT = NQL // W          # 4 query tiles
KC_Q = QD // 128       # 8
KC_C = CD // 128       # 6
NKC = NK // 128        # 8
HP = H // 2            # 8 head pairs


def build_bass():
    """Build the per-core Bass graph (SPMD, identical on all 8 cores)."""
    import concourse.bass as bass
    import concourse.tile as tile
    from concourse import bacc, mybir

    f32 = mybir.dt.float32
    bf16 = mybir.dt.bfloat16
    EXP = mybir.ActivationFunctionType.Exp

    nc = bacc.Bacc()

    xT_h = nc.dram_tensor("xT", (QD, NQL), bf16, kind="ExternalInput")
    ctxT_h = nc.dram_tensor("ctxT", (CD, NK), bf16, kind="ExternalInput")
    wq_h = nc.dram_tensor("wq", (QD, QD), bf16, kind="ExternalInput")
    wk_h = nc.dram_tensor("wk", (CD, QD), bf16, kind="ExternalInput")
    wv_h = nc.dram_tensor("wv", (CD, QD), bf16, kind="ExternalInput")
    wo_h = nc.dram_tensor("wo", (QD, QD), bf16, kind="ExternalInput")
    bo_h = nc.dram_tensor("bo", (1, QD), bf16, kind="ExternalInput")
    eye_h = nc.dram_tensor("eye", (128, 128), bf16, kind="ExternalInput")
    out_h = nc.dram_tensor("out", (NQL, QD), bf16, kind="ExternalOutput")

    xT_d = xT_h[:].rearrange("(c p) n -> p c n", p=128)       # [128, 8, 2048]
    ctxT_d = ctxT_h[:].rearrange("(c p) n -> p c n", p=128)   # [128, 6, 1024]
    wq_d = wq_h[:].rearrange("(c p) m -> p c m", p=128)
    wk_d = wk_h[:].rearrange("(c p) m -> p c m", p=128)
    wv_d = wv_h[:].rearrange("(c p) m -> p c m", p=128)
    wo_d = wo_h[:].rearrange("(c p) m -> p c m", p=128)
    out_d = out_h[:].rearrange("(t p) n -> p t n", p=128)     # [128, 16, 1024]

    with tile.TileContext(nc) as tc:
        _cms = []

        def open_pool(**kw):
            cm = tc.tile_pool(**kw)
            _cms.append(cm)
            return cm.__enter__()

        pp = open_pool(name="persist", bufs=1)
        # ---- persistent SBUF tiles
        qt_sb = pp.tile([128, KC_Q, 2, W], bf16)       # QT ring (per-tile)
        kt_sb = pp.tile([128, KC_Q, NK], bf16)         # KT   2 MB
        vp_sb = pp.tile([128, NKC, H, HD + 1], bf16)   # V'   2.08 MB
        attnT_sb = pp.tile([128, KC_Q, 2, W], bf16)    # attn ring (per-tile)
        wo_sb = pp.tile([128, KC_Q, QD], bf16)         # Wo   2 MB
        wq_sb = pp.tile([128, KC_Q, QD], bf16)         # Wq   2 MB
        # exp outputs: ring of 2 hp-buffers: slot index = (hp%2)*8 + kc
        etp = pp.tile([128, 16, 2, W], bf16)           # 4.2 MB
        bo_bc = pp.tile([128, QD], f32)
        ones_sb = pp.tile([1, 128], bf16)
        eye_sb = pp.tile([128, 128], bf16)

        # ---- PSUM (8 banks): A 4 + Bp 2 + ACC ring 2
        psS = open_pool(name="psS", bufs=1, space=bass.MemorySpace.PSUM)
        A = psS.tile([128, 2, 1024], f32)
        Bp = psS.tile([128, 1024], f32)
        accp = open_pool(name="accp", bufs=2, space=bass.MemorySpace.PSUM)

        # ---- small SBUF pools
        py = open_pool(name="yout", bufs=3)
        prs = open_pool(name="rsmall", bufs=2)
        pxq = open_pool(name="xq", bufs=2)
        pkv = open_pool(name="kvin", bufs=1)

        ctxT_sb = pkv.tile([128, KC_C, NK], bf16)      # 1.5 MB
        wk_sb = pkv.tile([128, KC_C, QD], bf16)        # 1.5 MB
        wv_sb = pkv.tile([128, KC_C, QD], bf16)        # 1.5 MB
        bo_sb = pkv.tile([1, QD], bf16)

        # copy engines for PSUM->SBUF moves; scalar only in the prefix
        _cp_idx = [0]

        def copy_out(dst, src, in_attention):
            # gpsimd cannot read PSUM; scalar must stay free during attention
            engs = (nc.vector,) if in_attention else (nc.vector, nc.scalar)
            eng = engs[_cp_idx[0] % len(engs)]
            _cp_idx[0] += 1
            if eng is nc.scalar:
                eng.copy(dst, src)
            else:
                eng.tensor_copy(dst, src)

        # ---------------- prefix ----------------
        # ctx/x/wq on the SP queue, wk/wv/bo/eye on the ACT queue: the
        # two DMA streams flow concurrently so K-proj's gate shrinks.
        nc.sync.dma_start(ctxT_sb[:, :, 0:512], ctxT_d[:, :, 0:512])
        nc.scalar.dma_start(wk_sb[:, :, 0:256], wk_d[:, :, 0:256])
        nc.scalar.dma_start(bo_sb[:], bo_h[:])
        nc.scalar.dma_start(eye_sb[:], eye_h[:])
        nc.sync.dma_start(ctxT_sb[:, :, 512:1024], ctxT_d[:, :, 512:1024])
        nc.scalar.dma_start(wk_sb[:, :, 256:1024], wk_d[:, :, 256:1024])

        xt0 = pxq.tile([128, KC_Q, W], bf16, tag="xt", name="xt0")

        nc.vector.memset(ones_sb[:], 1.0)
        nc.vector.memset(vp_sb[:, :, :, HD], 1.0)    # ones column per head
        for no in range(2):
            ps = accp.tile([128, 512], f32, tag="acc", name="psb")
            nc.tensor.matmul(ps[:], ones_sb[:],
                             bo_sb[0:1, no * 512:(no + 1) * 512],
                             start=True, stop=True)
            nc.scalar.copy(bo_bc[:, no * 512:(no + 1) * 512], ps[:])

        # ---- K projection (all keys; needed before any scores)
        for mo in range(KC_Q):
            for nk in range(2):
                ps = accp.tile([128, 512], f32, tag="acc", name="psk")
                for c in range(KC_C):
                    nc.tensor.matmul(
                        ps[:],
                        wk_sb[:, c, mo * 128:(mo + 1) * 128],
                        ctxT_sb[:, c, nk * 512:(nk + 1) * 512],
                        start=(c == 0), stop=(c == KC_C - 1),
                    )
                copy_out(kt_sb[:, mo, nk * 512:(nk + 1) * 512], ps[:], False)
            if mo == 0:
                nc.scalar.dma_start(wv_sb[:], wv_d)
                nc.sync.dma_start(xt0[:], xT_d[:, :, 0:W])
                nc.sync.dma_start(wq_sb[:], wq_d)

        def v_proj(ko, nv, in_attention):
            ps = accp.tile([128, 512], f32, tag="acc", name="psv")
            for c in range(KC_C):
                nc.tensor.matmul(
                    ps[:],
                    ctxT_sb[:, c, ko * 128:(ko + 1) * 128],
                    wv_sb[:, c, nv * 512:(nv + 1) * 512],
                    start=(c == 0), stop=(c == KC_C - 1),
                )
            copy_out(
                vp_sb[:, ko, nv * 8:(nv + 1) * 8, 0:HD],
                ps[:].rearrange("p (h d) -> p h d", h=8), in_attention)

        # ---- V projection for heads 0-7 (needed by tile0's early attn@V)
        for ko in range(NKC):
            v_proj(ko, 0, False)

        # ---- Q projection for tile 0
        for mo in range(KC_Q):
            ps = accp.tile([128, 512], f32, tag="acc", name="psq")
            for c in range(KC_Q):
                nc.tensor.matmul(
                    ps[:],
                    wq_sb[:, c, mo * 128:(mo + 1) * 128],
                    xt0[:, c, :],
                    start=(c == 0), stop=(c == KC_Q - 1),
                )
            copy_out(qt_sb[:, mo, 0, :], ps[:], False)

        nc.sync.dma_start(wo_sb[:], wo_d)

        # ---------- filler generators (yield once per matmul group) ----
        # V heads 8-15: drained during tile0 hp0-3, strictly before any
        # attn@V of hp>=4 (which is emitted at hp>=5) reads them.
        def v_group():
            for ko in range(NKC):
                v_proj(ko, 1, True)
                yield

        def q_group(t, xt):
            for mo in range(KC_Q):
                ps = accp.tile([128, 512], f32, tag="acc", name="psq2")
                for c in range(KC_Q):
                    nc.tensor.matmul(
                        ps[:],
                        wq_sb[:, c, mo * 128:(mo + 1) * 128],
                        xt[:, c, :],
                        start=(c == 0), stop=(c == KC_Q - 1),
                    )
                copy_out(qt_sb[:, mo, t % 2, :], ps[:], True)
                yield

        def y_group(t):
            for lo in range(4):
                mo = t * 4 + lo
                for no in range(2):
                    ps = accp.tile([128, 512], f32, tag="acc", name="psy")
                    for c in range(KC_Q):
                        nc.tensor.matmul(
                            ps[:],
                            attnT_sb[:, c, t % 2, lo * 128:(lo + 1) * 128],
                            wo_sb[:, c, no * 512:(no + 1) * 512],
                            start=(c == 0), stop=(c == KC_Q - 1),
                        )
                    y = py.tile([128, 512], bf16, tag="y")
                    nc.vector.tensor_add(
                        y[:], ps[:], bo_bc[:, no * 512:(no + 1) * 512])
                    nc.sync.dma_start(
                        out_d[:, mo, no * 512:(no + 1) * 512], y[:])
                    yield

        # ---------------- main attention loop ----------------
        pst = open_pool(name="stg", bufs=12)
        _stages = {}

        def attn_vO(t, hp, qcs):
            """attn@V in O[q,d] form + normalize into SBUF stage tiles.

            One PSUM slot holds both heads: O[:,0:65]=h0 (col 64 = softmax
            denominator), O[:,65:130]=h1.  Normalize with a per-partition
            reciprocal + tensor_scalar into stage[q,d] (bf16).  The PE
            transpose back to attnT's [d,q] layout happens one head-pair
            later (attn_vT) so the PE never waits on this DVE chain.
            """
            r = hp % 2
            for qc in qcs:
                O = accp.tile([128, 512], f32, tag="acc", name="O")
                for h_i in (0, 1):
                    h = 2 * hp + h_i
                    dst = O[:, h_i * 65:h_i * 65 + 65]
                    for kc in range(NKC):
                        nc.tensor.matmul(
                            dst,
                            etp[:, r * 8 + kc, h_i, qc * 128:(qc + 1) * 128],
                            vp_sb[:, kc, h, :],
                            start=(kc == 0), stop=(kc == NKC - 1),
                        )
                stage = pst.tile([128, 128], bf16, tag="st")
                for h_i in (0, 1):
                    rcp = prs.tile([128, 1], f32, tag="rcp")
                    nc.vector.reciprocal_approx_fast(
                        rcp[:], O[:, h_i * 65 + 64:h_i * 65 + 65])
                    nc.vector.tensor_scalar_mul(
                        stage[:, h_i * 64:(h_i + 1) * 64],
                        O[:, h_i * 65:h_i * 65 + 64], rcp[:])
                _stages[(t, hp, qc)] = stage

        def attn_vT(t, hp, qcs):
            for qc in qcs:
                stage = _stages.pop((t, hp, qc))
                tp = accp.tile([128, 512], f32, tag="acc", name="tp")
                tp16 = tp[:, 0:64].bitcast(bf16)
                nc.tensor.transpose(tp16, stage[:], eye_sb[:])
                nc.vector.tensor_copy(
                    attnT_sb[:, hp, t % 2, qc * 128:(qc + 1) * 128], tp16)

        pending = None          # (t, hp) owed attn@V O/normalize
        pending2 = None         # (t, hp) owed transposes
        fillers = []

        def take_filler(n):
            # drain sequentially: first generator until exhausted, then next
            for _ in range(n):
                while fillers:
                    if next(fillers[0], "DONE") == "DONE":
                        fillers.pop(0)
                        continue
                    break

        # y(t-1) must not drain at hp0: its c=7 matmul needs the transpose
        # of (t-1, hp7) which is only issued while hp0 runs.
        N_FILL = [0, 2, 2, 2, 2, 2, 3, 3]

        for t in range(NT):
            if t == 0:
                fillers.append(v_group())
            else:
                fillers.append(y_group(t - 1))
            if t + 1 < NT:
                xt = pxq.tile([128, KC_Q, W], bf16, tag="xt",
                              name=f"xt{t + 1}")
                nc.sync.dma_start(xt[:], xT_d[:, :, (t + 1) * W:(t + 2) * W])
                fillers.append(q_group(t + 1, xt))

            for hp in range(HP):
                r = hp % 2

                def spair(kc, dst_h0, dst_h1):
                    ks = slice(kc * 128, (kc + 1) * 128)
                    nc.tensor.matmul(
                        dst_h0, kt_sb[0:64, hp, ks],
                        qt_sb[0:64, hp, t % 2, :],
                        start=True, stop=True, tile_position=(0, 0),
                    )
                    nc.tensor.matmul(
                        dst_h1, kt_sb[64:128, hp, ks],
                        qt_sb[64:128, hp, t % 2, :],
                        start=True, stop=True, tile_position=(64, 0),
                    )

                def exp_a(kc):
                    nc.scalar.activation(
                        etp[:, r * 8 + kc:r * 8 + kc + 2, :, :],
                        A[:], EXP, scale=SCALE)

                def exp_b(kc):
                    nc.scalar.activation(
                        etp[:, r * 8 + kc:r * 8 + kc + 1, :, :],
                        Bp[:], EXP, scale=SCALE)

                # kc pattern: {0,1}->A, 2->B, {3,4}->A, 5->B, {6,7}->A
                spair(0, A[:, 0, 0:512], A[:, 0, 512:1024])
                spair(1, A[:, 1, 0:512], A[:, 1, 512:1024])
                exp_a(0)
                spair(2, Bp[:, 0:512], Bp[:, 512:1024])
                exp_b(2)
                if pending2 is not None:
                    attn_vT(*pending2, (0, 1, 2, 3))
                if pending is not None:
                    attn_vO(*pending, (0, 1))
                spair(3, A[:, 0, 0:512], A[:, 0, 512:1024])
                spair(4, A[:, 1, 0:512], A[:, 1, 512:1024])
                exp_a(3)
                if pending is not None:
                    attn_vO(*pending, (2,))
                spair(5, Bp[:, 0:512], Bp[:, 512:1024])
                exp_b(5)
                if pending is not None:
                    attn_vO(*pending, (3,))
                spair(6, A[:, 0, 0:512], A[:, 0, 512:1024])
                spair(7, A[:, 1, 0:512], A[:, 1, 512:1024])
                exp_a(6)
                take_filler(N_FILL[hp])
                pending2 = pending
                pending = (t, hp)

        # drain: last two head-pairs' attn@V stages + last y tile
        attn_vO(*pending, (0, 1, 2, 3))
        attn_vT(*pending2, (0, 1, 2, 3))
        attn_vT(*pending, (0, 1, 2, 3))
        take_filler(1000)
        for _ in y_group(NT - 1):
            pass

        for cm in reversed(_cms):
            cm.__exit__(None, None, None)

    nc.finalize()
    return nc


def make_in_maps(x, context, Wq, Wk, Wv, Wo, bo):
    """Host-side sharding + layout prep: transpose and cast to bf16."""
    import ml_dtypes
    bf16 = ml_dtypes.bfloat16

    x = np.asarray(x, np.float32)
    context = np.asarray(context, np.float32)
    wq = np.asarray(Wq, np.float32).astype(bf16)
    wk = np.asarray(Wk, np.float32).astype(bf16)
    wv = np.asarray(Wv, np.float32).astype(bf16)
    wo = np.asarray(Wo, np.float32).astype(bf16)
    bo = np.asarray(bo, np.float32).reshape(1, QD).astype(bf16)
    eye = np.eye(128, dtype=np.float32).astype(bf16)

    in_maps = []
    for c in range(N_CORES):
        b, half = c // 2, c % 2
        xs = x[b, half * NQL:(half + 1) * NQL, :]           # [2048, 1024]
        in_maps.append({
            "xT": np.ascontiguousarray(xs.T).astype(bf16),   # [1024, 2048]
            "ctxT": np.ascontiguousarray(context[b].T).astype(bf16),
            "wq": wq, "wk": wk, "wv": wv, "wo": wo, "bo": bo, "eye": eye,
        })
    return in_maps


_NC_CACHE = {}


def kernel(x, context, Wq, Wk, Wv, Wo, bo, _trace=False):
    import sys
    if "/opt/trn_rl_repo" not in sys.path:
        sys.path.insert(0, "/opt/trn_rl_repo")
    from concourse.bass_utils import run_bass_kernel_spmd

    if "nc" not in _NC_CACHE:
        _NC_CACHE["nc"] = build_bass()
    nc = _NC_CACHE["nc"]

    in_maps = make_in_maps(x, context, Wq, Wk, Wv, Wo, bo)
    res = run_bass_kernel_spmd(
        nc, in_maps, core_ids=list(range(N_CORES)), trace=_trace)

    out = np.empty((B, NQ, QD), np.float32)
    for c in range(N_CORES):
        b, half = c // 2, c % 2
        out[b, half * NQL:(half + 1) * NQL, :] = np.asarray(
            res.results[c]["out"]).astype(np.float32)
    if _trace:
        return out, res
    return out
